# revision 34
# baseline (speedup 1.0000x reference)
"""Trainium2 Bass kernel for NetworksPlusCircuit.

Two MLPs (784->512->10, relu, softmax) over B=65536 samples each, then
P(sum=s) = sum_{a+c=s} p0[a]*p1[c]  -> [B, 19].

Sharding: pure data parallel over the batch across 8 NeuronCores.
Host-side prep: shard + transpose x to [784, B/8] per core so the device
DMA streams are contiguous; weights replicated (cast to bf16 on host).

Device-side per core (BC = 8192 samples per net):
  per round (512 samples of one net):
    - SWDGE cast-DMA loads xT tile [128f, 7fc, 512b] (fp32 HBM -> bf16 SBUF)
    - mm1: hT[j, b] += W1[f, j].T @ xT[f, b]   (PSUM, 4 j-chunks x 7 f-chunks)
    - bias+relu+cast -> SBUF bf16 (split across DVE and ACT)
    - mm2: l[b, 10]  += hT[j, b].T @ W2[j, 10] (PSUM, accumulate 4 j-chunks)
    - exp (ACT, PSUM -> SBUF block buffer), softmax denominator deferred
  per block (8 pair-rounds = 4096 sample pairs):
    - scale by exp(b2), Z sums, digit-sum convolution, normalize (DVE), DMA out
"""

import sys
import types

import numpy as np
import concourse.bass as bass
import concourse.bacc as bacc
import concourse.mybir as mybir
import concourse.tile as tile
from concourse.bass_utils import run_bass_kernel_spmd

F32 = mybir.dt.float32
MMDT = mybir.dt.float16  # matmul dtype: fp16 = bf16 speed, 4x less rounding error

NCORES = 8
B = 65536
BC = B // NCORES            # samples per core per net
F = 784                     # input features
HID = 512
NCLS = 10
NSUM = 19
FCH_FULL = F // 128         # 6 full feature chunks
FT = F - FCH_FULL * 128     # 16 tail features
FCH = FCH_FULL + 1          # 7 chunks
JCH = HID // 128            # 4 hidden chunks
# The 16 tail features are replicated at 4 32-row strips (rows 768+32k..+16)
# so the 4 per-j-chunk tail matmuls sit in distinct PE row-groups and run
# concurrently (row tiling).
FPAD = FCH * 128  # padded rows: 6*128 full chunks + 4 strips * 32 = 896
RN = 512                    # samples per compute round
XR = 1024                   # samples per x-load tile
XRR = XR // RN              # compute rounds per x tile
NR = BC // RN               # 16 pair-rounds per core
NBLK = 8                    # conv blocks; NR % NBLK == 0
RPB = NR // NBLK            # pair-rounds per block
GPB = RPB * (RN // 128)     # 128-sample groups per block


def pad_rows(a, dtype):
    """[F, ...] -> [FPAD, ...] with tail rows replicated at 4 32-row strips."""
    out = np.zeros((FPAD,) + a.shape[1:], dtype=dtype)
    out[: FCH_FULL * 128] = a[: FCH_FULL * 128]
    for k in range(JCH):
        base = FCH_FULL * 128 + 32 * k
        out[base : base + FT] = a[FCH_FULL * 128 :]
    return out


def build_nc():
    nc = bacc.Bacc("TRN2", target_bir_lowering=False, debug=False, num_devices=NCORES)

    xt = [
        nc.dram_tensor(f"xt{n}", [FPAD, BC], F32, kind="ExternalInput")
        for n in range(2)
    ]
    # weights arrive host-prepacked in their on-chip layouts so every const
    # DMA is one contiguous line per partition (cheap descriptor generation)
    w1 = [
        nc.dram_tensor(f"w1_{n}", [128, FCH, HID], MMDT, kind="ExternalInput")
        for n in range(2)
    ]
    w2 = [
        nc.dram_tensor(f"w2_{n}", [128, JCH, NCLS], MMDT, kind="ExternalInput")
        for n in range(2)
    ]
    # cpack[p] = [b1_0 (4), b1_1 (4), eb2_0 (10), eb2_1 (10)]
    cpk = nc.dram_tensor("cpack", [128, 28], F32, kind="ExternalInput")
    out = nc.dram_tensor("out", [BC, NSUM], F32, kind="ExternalOutput")

    with tile.TileContext(nc) as tc:
        with (
            tc.tile_pool(name="consts", bufs=1) as consts,
            tc.tile_pool(name="xt", bufs=4) as xt_pool,
            tc.tile_pool(name="ht", bufs=3) as ht_pool,
            tc.tile_pool(name="eblk", bufs=3) as e_pool,
            tc.tile_pool(name="conv", bufs=1) as conv_pool,
            tc.tile_pool(name="outp", bufs=2) as out_pool,
            tc.tile_pool(name="hpsum", bufs=6, space="PSUM") as h_pool,
            tc.tile_pool(name="lpsum", bufs=2, space="PSUM") as l_pool,
        ):
            # ---- constants (HWDGE; identity layouts) ----
            w1_sb = []
            w2_sb = []
            for n in range(2):
                w = consts.tile([128, FCH, HID], MMDT, name=f"w1sb{n}", tag=f"w1sb{n}")
                nc.sync.dma_start(out=w, in_=w1[n].ap())
                w1_sb.append(w)
            for n in range(2):
                w2t = consts.tile([128, JCH, NCLS], MMDT, name=f"w2sb{n}", tag=f"w2sb{n}")
                nc.sync.dma_start(out=w2t, in_=w2[n].ap())
                w2_sb.append(w2t)
            cpack = consts.tile([128, 28], F32, name="cpack", tag="cpack")
            nc.sync.dma_start(out=cpack, in_=cpk.ap())
            b1_sb = [cpack[:, 0:JCH], cpack[:, JCH : 2 * JCH]]
            eb2_sb = [
                cpack[:, 2 * JCH : 2 * JCH + NCLS],
                cpack[:, 2 * JCH + NCLS : 2 * JCH + 2 * NCLS],
            ]

            # ---- main loop ----
            xtiles = {}
            for blk in range(NBLK):
                # exp values for this block: [128, net, group, cls]
                e_blk = e_pool.tile([128, 2, GPB, NCLS], F32, name="eblk", tag="eblk")

                for rr in range(RPB):
                    r = blk * RPB + rr
                    for n in range(2):
                        # x tiles hold XR samples (several compute rounds):
                        # bigger DMA lines (XR*4B per partition) + fewer DMAs
                        if r % XRR == 0:
                            xtiles[n] = xt_pool.tile(
                                [128, FCH, XR], MMDT, name="xt", tag="xt"
                            )
                            xsrc = xt[n].ap()[:, r * RN : r * RN + XR]
                            # tail chunk (6) first: each compute round begins
                            # with the row-tiled tail matmuls
                            for c0, c1 in ((6, 7), (0, 2), (2, 4), (4, 6)):
                                nc.gpsimd.dma_start(
                                    out=xtiles[n][:, c0:c1, :],
                                    in_=xsrc[c0 * 128 : c1 * 128, :].rearrange(
                                        "(c p) b -> p c b", p=128
                                    ),
                                )
                        xtile = xtiles[n][:, :, (r % XRR) * RN : (r % XRR + 1) * RN]

                        ph = [
                            h_pool.tile([128, RN], F32, name="ph", tag="ph")
                            for i in range(JCH)
                        ]
                        # tail features first: 4 concurrent row-tiled K=16
                        # matmuls (start=True initializes every psum element)
                        for jc in range(JCH):
                            p0 = 32 * jc
                            nc.tensor.matmul(
                                out=ph[jc][:, :],
                                lhsT=w1_sb[n][
                                    p0 : p0 + FT, FCH_FULL, jc * 128 : (jc + 1) * 128
                                ],
                                rhs=xtile[p0 : p0 + FT, FCH_FULL, :],
                                start=True,
                                stop=False,
                                tile_position=(p0, 0),
                            )
                        # jc-major: each j-chunk finishes early so its relu
                        # can drain the psum slot while the round continues
                        for jc in range(JCH):
                            for fc in range(FCH_FULL):
                                nc.tensor.matmul(
                                    out=ph[jc][:, :],
                                    lhsT=w1_sb[n][:, fc, jc * 128 : (jc + 1) * 128],
                                    rhs=xtile[:, fc, :],
                                    start=False,
                                    stop=(fc == FCH_FULL - 1),
                                )

                        ht = ht_pool.tile([128, JCH, RN], MMDT, name="ht", tag="ht")
                        for jc in range(JCH):
                            # jc3 finishes last and gates mm2: run it on DVE in
                            # parallel with ACT's relus; rest on ACT so DVE
                            # stays mostly free for the conv blocks
                            if jc == JCH - 1:
                                nc.vector.tensor_scalar(
                                    out=ht[:, jc, :],
                                    in0=ph[jc][:, :],
                                    scalar1=b1_sb[n][:, jc : jc + 1],
                                    scalar2=0.0,
                                    op0=mybir.AluOpType.add,
                                    op1=mybir.AluOpType.max,
                                )
                            else:
                                nc.scalar.activation(
                                    out=ht[:, jc, :],
                                    in_=ph[jc][:, :],
                                    func=mybir.ActivationFunctionType.Relu,
                                    bias=b1_sb[n][:, jc : jc + 1],
                                    scale=1.0,
                                )

                        pl = l_pool.tile([128, RN // 128, NCLS], F32, name="pl", tag="pl")
                        for bc in range(RN // 128):
                            for jc in range(JCH):
                                nc.tensor.matmul(
                                    out=pl[:, bc, :],
                                    lhsT=ht[:, jc, bc * 128 : (bc + 1) * 128],
                                    rhs=w2_sb[n][:, jc, :],
                                    start=(jc == 0),
                                    stop=(jc == JCH - 1),
                                )

                        nc.scalar.activation(
                            out=e_blk[:, n, rr * 4 : (rr + 1) * 4, :],
                            in_=pl[:, :, :],
                            func=mybir.ActivationFunctionType.Exp,
                        )

                # ---- conv over a range of groups ----
                def emit_conv(ev, gc, row0):
                    # ev: e slice [128, 2, gc, NCLS]; writes out rows row0..
                    for n in range(2):
                        nc.vector.tensor_tensor(
                            out=ev[:, n, :, :],
                            in0=ev[:, n, :, :],
                            in1=eb2_sb[n][:, None, :].to_broadcast([128, gc, NCLS]),
                            op=mybir.AluOpType.mult,
                        )
                    z = conv_pool.tile([128, 2, gc], F32, name="z", tag="z")
                    for n in range(2):
                        nc.vector.reduce_sum(
                            out=z[:, n, :],
                            in_=ev[:, n, :, :],
                            axis=mybir.AxisListType.X,
                        )
                    rz = conv_pool.tile([128, gc], F32, name="rz", tag="rz")
                    nc.vector.tensor_tensor(
                        out=rz, in0=z[:, 0, :], in1=z[:, 1, :], op=mybir.AluOpType.mult
                    )
                    nc.vector.reciprocal(out=rz, in_=rz)
                    prods = conv_pool.tile(
                        [128, gc, NCLS, NCLS], F32, name="prods", tag="prods"
                    )
                    for a in range(NCLS):
                        nc.vector.tensor_tensor(
                            out=prods[:, :, a, :],
                            in0=ev[:, 1, :, :],
                            in1=ev[:, 0, :, a : a + 1].to_broadcast([128, gc, NCLS]),
                            op=mybir.AluOpType.mult,
                        )
                    acc = out_pool.tile([128, gc, NSUM], F32, name="acc", tag="acc")
                    nc.vector.memset(acc, 0.0)
                    for a in range(NCLS):
                        nc.vector.tensor_tensor(
                            out=acc[:, :, a : a + NCLS],
                            in0=acc[:, :, a : a + NCLS],
                            in1=prods[:, :, a, :],
                            op=mybir.AluOpType.add,
                        )
                    nc.vector.tensor_tensor(
                        out=acc,
                        in0=acc,
                        in1=rz[:, :, None].to_broadcast([128, gc, NSUM]),
                        op=mybir.AluOpType.mult,
                    )
                    nc.sync.dma_start(
                        out=out.ap()[row0 : row0 + gc * 128, :].rearrange(
                            "(g p) s -> p g s", p=128
                        ),
                        in_=acc,
                    )

                if blk == NBLK - 1 and GPB % 2 == 0:
                    # final block in halves: less conv exposed after last matmul
                    hg = GPB // 2
                    emit_conv(e_blk[:, :, 0:hg, :], hg, blk * GPB * 128)
                    emit_conv(
                        e_blk[:, :, hg:GPB, :], hg, blk * GPB * 128 + hg * 128
                    )
                else:
                    emit_conv(e_blk, GPB, blk * GPB * 128)

    nc.compile()
    return nc


_NC_CACHE = {}


def _get_nc():
    if "nc" not in _NC_CACHE:
        _NC_CACHE["nc"] = build_nc()
    return _NC_CACHE["nc"]


def _install_ntff_hook():
    """Shim antenv.axon_hooks (absent in this image) so trace=True can work."""
    try:
        import antenv

        if hasattr(antenv, "axon_hooks"):
            return
        from trn_agent_boot.trn_boot import _ntff_profile_via_ctypes

        mod = types.ModuleType("antenv.axon_hooks")
        holder = {"hook": _ntff_profile_via_ctypes("/opt/axon/libaxon_pjrt.so")}
        mod.set_axon_ntff_profile_hook = lambda h: holder.__setitem__("hook", h)
        mod.get_axon_ntff_profile_hook = lambda: holder["hook"]
        sys.modules["antenv.axon_hooks"] = mod
        antenv.axon_hooks = mod
    except Exception:
        pass


def kernel(x, W1_0, b1_0, W2_0, b2_0, W1_1, b1_1, W2_1, b2_1, _trace=False):
    x = np.asarray(x, dtype=np.float32)

    xf = x.reshape(2, B, F)
    weights = {}
    cpack = np.zeros((128, 28), np.float32)
    for n, (W1n, b1n, W2n, b2n) in enumerate(
        [(W1_0, b1_0, W2_0, b2_0), (W1_1, b1_1, W2_1, b2_1)]
    ):
        w1p = pad_rows(np.asarray(W1n, np.float32).astype(np.float16), np.float16)
        weights[f"w1_{n}"] = np.ascontiguousarray(
            w1p.reshape(FCH, 128, HID).transpose(1, 0, 2)
        )
        w2 = np.asarray(W2n, np.float32).astype(np.float16)
        weights[f"w2_{n}"] = np.ascontiguousarray(
            w2.reshape(JCH, 128, NCLS).transpose(1, 0, 2)
        )
        cpack[:, n * JCH : (n + 1) * JCH] = (
            np.asarray(b1n, np.float32).reshape(JCH, 128).T
        )
        cpack[:, 2 * JCH + n * NCLS : 2 * JCH + (n + 1) * NCLS] = np.exp(
            np.asarray(b2n, np.float32)
        )[None, :]
    weights["cpack"] = cpack

    in_maps = []
    for c in range(NCORES):
        m = dict(weights)
        for n in range(2):
            shard = xf[n, c * BC : (c + 1) * BC, :]  # [BC, F]
            m[f"xt{n}"] = pad_rows(np.ascontiguousarray(shard.T), np.float32)
        in_maps.append(m)

    nc = _get_nc()
    if _trace:
        _install_ntff_hook()
    res = run_bass_kernel_spmd(nc, in_maps, list(range(NCORES)), trace=_trace)
    pieces = [res.results[c]["out"] for c in range(NCORES)]
    full = np.concatenate(pieces, axis=0).astype(np.float32)
    if _trace:
        return full, res
    return full


# revision 35
# speedup vs baseline: 1.2428x; 1.2428x over previous
"""Trainium2 Bass kernel for NetworksPlusCircuit.

Two MLPs (784->512->10, relu, softmax) over B=65536 samples each, then
P(sum=s) = sum_{a+c=s} p0[a]*p1[c]  -> [B, 19].

Sharding: pure data parallel over the batch across 8 NeuronCores.
Host-side prep: shard + transpose x to [784, B/8] per core so the device
DMA streams are contiguous; weights replicated (cast to bf16 on host).

Device-side per core (BC = 8192 samples per net):
  per round (512 samples of one net):
    - SWDGE cast-DMA loads xT tile [128f, 7fc, 512b] (fp32 HBM -> bf16 SBUF)
    - mm1: hT[j, b] += W1[f, j].T @ xT[f, b]   (PSUM, 4 j-chunks x 7 f-chunks)
    - bias+relu+cast -> SBUF bf16 (split across DVE and ACT)
    - mm2: l[b, 10]  += hT[j, b].T @ W2[j, 10] (PSUM, accumulate 4 j-chunks)
    - exp (ACT, PSUM -> SBUF block buffer), softmax denominator deferred
  per block (8 pair-rounds = 4096 sample pairs):
    - scale by exp(b2), Z sums, digit-sum convolution, normalize (DVE), DMA out
"""

import sys
import types

import numpy as np
import concourse.bass as bass
import concourse.bacc as bacc
import concourse.mybir as mybir
import concourse.tile as tile
from concourse.bass_utils import run_bass_kernel_spmd

F32 = mybir.dt.float32
MMDT = mybir.dt.float16  # matmul dtype: fp16 = bf16 speed, 4x less rounding error

NCORES = 8
B = 65536
BC = B // NCORES            # samples per core per net
F = 784                     # input features
HID = 512
NCLS = 10
NSUM = 19
FCH_FULL = F // 128         # 6 full feature chunks
FT = F - FCH_FULL * 128     # 16 tail features
FCH = FCH_FULL + 1          # 7 chunks
JCH = HID // 128            # 4 hidden chunks
# The 16 tail features are replicated at 4 32-row strips (rows 768+32k..+16)
# so the 4 per-j-chunk tail matmuls sit in distinct PE row-groups and run
# concurrently (row tiling).
FPAD = FCH * 128  # padded rows: 6*128 full chunks + 4 strips * 32 = 896
RN = 512                    # samples per compute round
XR = 1024                   # samples per x-load tile
XRR = XR // RN              # compute rounds per x tile
NR = BC // RN               # 16 pair-rounds per core
NBLK = 8                    # conv blocks; NR % NBLK == 0
RPB = NR // NBLK            # pair-rounds per block
GPB = RPB * (RN // 128)     # 128-sample groups per block


def pad_rows(a, dtype):
    """[F, ...] -> [FPAD, ...] with tail rows replicated at 4 32-row strips."""
    out = np.zeros((FPAD,) + a.shape[1:], dtype=dtype)
    out[: FCH_FULL * 128] = a[: FCH_FULL * 128]
    for k in range(JCH):
        base = FCH_FULL * 128 + 32 * k
        out[base : base + FT] = a[FCH_FULL * 128 :]
    return out


def build_nc():
    nc = bacc.Bacc("TRN2", target_bir_lowering=False, debug=False, num_devices=NCORES)

    xt = [
        nc.dram_tensor(f"xt{n}", [FPAD, BC], F32, kind="ExternalInput")
        for n in range(2)
    ]
    # weights arrive host-prepacked in their on-chip layouts so every const
    # DMA is one contiguous line per partition (cheap descriptor generation)
    w1 = [
        nc.dram_tensor(f"w1_{n}", [128, FCH, HID], MMDT, kind="ExternalInput")
        for n in range(2)
    ]
    w2 = [
        nc.dram_tensor(f"w2_{n}", [128, JCH, NCLS], MMDT, kind="ExternalInput")
        for n in range(2)
    ]
    # cpack[p] = [b1_0 (4), b1_1 (4), eb2_0 (10), eb2_1 (10)]
    cpk = nc.dram_tensor("cpack", [128, 28], F32, kind="ExternalInput")
    out = nc.dram_tensor("out", [BC, NSUM], F32, kind="ExternalOutput")

    with tile.TileContext(nc) as tc:
        with (
            tc.tile_pool(name="consts", bufs=1) as consts,
            tc.tile_pool(name="xt", bufs=4) as xt_pool,
            tc.tile_pool(name="ht", bufs=3) as ht_pool,
            tc.tile_pool(name="eblk", bufs=3) as e_pool,
            tc.tile_pool(name="conv", bufs=1) as conv_pool,
            tc.tile_pool(name="outp", bufs=2) as out_pool,
            tc.tile_pool(name="hpsum", bufs=6, space="PSUM") as h_pool,
            tc.tile_pool(name="lpsum", bufs=2, space="PSUM") as l_pool,
        ):
            # ---- constants (HWDGE; identity layouts) ----
            w1_sb = []
            w2_sb = []
            for n in range(2):
                w = consts.tile([128, FCH, HID], MMDT, name=f"w1sb{n}", tag=f"w1sb{n}")
                nc.sync.dma_start(out=w, in_=w1[n].ap())
                w1_sb.append(w)
            for n in range(2):
                w2t = consts.tile([128, JCH, NCLS], MMDT, name=f"w2sb{n}", tag=f"w2sb{n}")
                nc.sync.dma_start(out=w2t, in_=w2[n].ap())
                w2_sb.append(w2t)
            cpack = consts.tile([128, 28], F32, name="cpack", tag="cpack")
            nc.sync.dma_start(out=cpack, in_=cpk.ap())
            b1_sb = [cpack[:, 0:JCH], cpack[:, JCH : 2 * JCH]]
            eb2_sb = [
                cpack[:, 2 * JCH : 2 * JCH + NCLS],
                cpack[:, 2 * JCH + NCLS : 2 * JCH + 2 * NCLS],
            ]

            # ---- main loop ----
            xtiles = {}
            for blk in range(NBLK):
                # exp values for this block: [128, net, group, cls]
                e_blk = e_pool.tile([128, 2, GPB, NCLS], F32, name="eblk", tag="eblk")

                for rr in range(RPB):
                    r = blk * RPB + rr
                    for n in range(2):
                        # x tiles hold XR samples (several compute rounds):
                        # bigger DMA lines (XR*4B per partition) + fewer DMAs
                        if r % XRR == 0:
                            xtiles[n] = xt_pool.tile(
                                [128, FCH, XR], MMDT, name="xt", tag="xt"
                            )
                            xsrc = xt[n].ap()[:, r * RN : r * RN + XR]
                            # tail chunk (6) first: each compute round begins
                            # with the row-tiled tail matmuls
                            for c0, c1 in ((6, 7), (0, 2), (2, 4), (4, 6)):
                                nc.gpsimd.dma_start(
                                    out=xtiles[n][:, c0:c1, :],
                                    in_=xsrc[c0 * 128 : c1 * 128, :].rearrange(
                                        "(c p) b -> p c b", p=128
                                    ),
                                )
                        xtile = xtiles[n][:, :, (r % XRR) * RN : (r % XRR + 1) * RN]

                        ph = [
                            h_pool.tile([128, RN], F32, name="ph", tag="ph")
                            for i in range(JCH)
                        ]
                        # tail features first: 4 concurrent row-tiled K=16
                        # matmuls (start=True initializes every psum element)
                        for jc in range(JCH):
                            p0 = 32 * jc
                            nc.tensor.matmul(
                                out=ph[jc][:, :],
                                lhsT=w1_sb[n][
                                    p0 : p0 + FT, FCH_FULL, jc * 128 : (jc + 1) * 128
                                ],
                                rhs=xtile[p0 : p0 + FT, FCH_FULL, :],
                                start=True,
                                stop=False,
                                tile_position=(p0, 0),
                            )
                        # jc-major: each j-chunk finishes early so its relu
                        # can drain the psum slot while the round continues
                        for jc in range(JCH):
                            for fc in range(FCH_FULL):
                                nc.tensor.matmul(
                                    out=ph[jc][:, :],
                                    lhsT=w1_sb[n][:, fc, jc * 128 : (jc + 1) * 128],
                                    rhs=xtile[:, fc, :],
                                    start=False,
                                    stop=(fc == FCH_FULL - 1),
                                )

                        ht = ht_pool.tile([128, JCH, RN], MMDT, name="ht", tag="ht")
                        for jc in range(JCH):
                            # all on ACT: DVE must stay free for the conv
                            # blocks or PSUM drains stall and starve the PE
                            nc.scalar.activation(
                                out=ht[:, jc, :],
                                in_=ph[jc][:, :],
                                func=mybir.ActivationFunctionType.Relu,
                                bias=b1_sb[n][:, jc : jc + 1],
                                scale=1.0,
                            )

                        pl = l_pool.tile([128, RN // 128, NCLS], F32, name="pl", tag="pl")
                        for bc in range(RN // 128):
                            for jc in range(JCH):
                                nc.tensor.matmul(
                                    out=pl[:, bc, :],
                                    lhsT=ht[:, jc, bc * 128 : (bc + 1) * 128],
                                    rhs=w2_sb[n][:, jc, :],
                                    start=(jc == 0),
                                    stop=(jc == JCH - 1),
                                )

                        nc.scalar.activation(
                            out=e_blk[:, n, rr * 4 : (rr + 1) * 4, :],
                            in_=pl[:, :, :],
                            func=mybir.ActivationFunctionType.Exp,
                        )

                # ---- conv over a range of groups ----
                def emit_conv(ev, gc, row0):
                    # ev: e slice [128, 2, gc, NCLS]; writes out rows row0..
                    for n in range(2):
                        nc.vector.tensor_tensor(
                            out=ev[:, n, :, :],
                            in0=ev[:, n, :, :],
                            in1=eb2_sb[n][:, None, :].to_broadcast([128, gc, NCLS]),
                            op=mybir.AluOpType.mult,
                        )
                    z = conv_pool.tile([128, 2, gc], F32, name="z", tag="z")
                    for n in range(2):
                        nc.vector.reduce_sum(
                            out=z[:, n, :],
                            in_=ev[:, n, :, :],
                            axis=mybir.AxisListType.X,
                        )
                    rz = conv_pool.tile([128, gc], F32, name="rz", tag="rz")
                    nc.vector.tensor_tensor(
                        out=rz, in0=z[:, 0, :], in1=z[:, 1, :], op=mybir.AluOpType.mult
                    )
                    nc.vector.reciprocal(out=rz, in_=rz)
                    prods = conv_pool.tile(
                        [128, gc, NCLS, NCLS], F32, name="prods", tag="prods"
                    )
                    for a in range(NCLS):
                        nc.vector.tensor_tensor(
                            out=prods[:, :, a, :],
                            in0=ev[:, 1, :, :],
                            in1=ev[:, 0, :, a : a + 1].to_broadcast([128, gc, NCLS]),
                            op=mybir.AluOpType.mult,
                        )
                    acc = out_pool.tile([128, gc, NSUM], F32, name="acc", tag="acc")
                    nc.vector.memset(acc, 0.0)
                    for a in range(NCLS):
                        nc.vector.tensor_tensor(
                            out=acc[:, :, a : a + NCLS],
                            in0=acc[:, :, a : a + NCLS],
                            in1=prods[:, :, a, :],
                            op=mybir.AluOpType.add,
                        )
                    nc.vector.tensor_tensor(
                        out=acc,
                        in0=acc,
                        in1=rz[:, :, None].to_broadcast([128, gc, NSUM]),
                        op=mybir.AluOpType.mult,
                    )
                    nc.sync.dma_start(
                        out=out.ap()[row0 : row0 + gc * 128, :].rearrange(
                            "(g p) s -> p g s", p=128
                        ),
                        in_=acc,
                    )

                if blk == NBLK - 1 and GPB % 2 == 0:
                    # final block in halves: less conv exposed after last matmul
                    hg = GPB // 2
                    emit_conv(e_blk[:, :, 0:hg, :], hg, blk * GPB * 128)
                    emit_conv(
                        e_blk[:, :, hg:GPB, :], hg, blk * GPB * 128 + hg * 128
                    )
                else:
                    emit_conv(e_blk, GPB, blk * GPB * 128)

    nc.compile()
    return nc


_NC_CACHE = {}


def _get_nc():
    if "nc" not in _NC_CACHE:
        _NC_CACHE["nc"] = build_nc()
    return _NC_CACHE["nc"]


def _install_ntff_hook():
    """Shim antenv.axon_hooks (absent in this image) so trace=True can work."""
    try:
        import antenv

        if hasattr(antenv, "axon_hooks"):
            return
        from trn_agent_boot.trn_boot import _ntff_profile_via_ctypes

        mod = types.ModuleType("antenv.axon_hooks")
        holder = {"hook": _ntff_profile_via_ctypes("/opt/axon/libaxon_pjrt.so")}
        mod.set_axon_ntff_profile_hook = lambda h: holder.__setitem__("hook", h)
        mod.get_axon_ntff_profile_hook = lambda: holder["hook"]
        sys.modules["antenv.axon_hooks"] = mod
        antenv.axon_hooks = mod
    except Exception:
        pass


def kernel(x, W1_0, b1_0, W2_0, b2_0, W1_1, b1_1, W2_1, b2_1, _trace=False):
    x = np.asarray(x, dtype=np.float32)

    xf = x.reshape(2, B, F)
    weights = {}
    cpack = np.zeros((128, 28), np.float32)
    for n, (W1n, b1n, W2n, b2n) in enumerate(
        [(W1_0, b1_0, W2_0, b2_0), (W1_1, b1_1, W2_1, b2_1)]
    ):
        w1p = pad_rows(np.asarray(W1n, np.float32).astype(np.float16), np.float16)
        weights[f"w1_{n}"] = np.ascontiguousarray(
            w1p.reshape(FCH, 128, HID).transpose(1, 0, 2)
        )
        w2 = np.asarray(W2n, np.float32).astype(np.float16)
        weights[f"w2_{n}"] = np.ascontiguousarray(
            w2.reshape(JCH, 128, NCLS).transpose(1, 0, 2)
        )
        cpack[:, n * JCH : (n + 1) * JCH] = (
            np.asarray(b1n, np.float32).reshape(JCH, 128).T
        )
        cpack[:, 2 * JCH + n * NCLS : 2 * JCH + (n + 1) * NCLS] = np.exp(
            np.asarray(b2n, np.float32)
        )[None, :]
    weights["cpack"] = cpack

    in_maps = []
    for c in range(NCORES):
        m = dict(weights)
        for n in range(2):
            shard = xf[n, c * BC : (c + 1) * BC, :]  # [BC, F]
            m[f"xt{n}"] = pad_rows(np.ascontiguousarray(shard.T), np.float32)
        in_maps.append(m)

    nc = _get_nc()
    if _trace:
        _install_ntff_hook()
    res = run_bass_kernel_spmd(nc, in_maps, list(range(NCORES)), trace=_trace)
    pieces = [res.results[c]["out"] for c in range(NCORES)]
    full = np.concatenate(pieces, axis=0).astype(np.float32)
    if _trace:
        return full, res
    return full


# revision 36
# speedup vs baseline: 1.2936x; 1.0409x over previous
"""Trainium2 Bass kernel for NetworksPlusCircuit.

Two MLPs (784->512->10, relu, softmax) over B=65536 samples each, then
P(sum=s) = sum_{a+c=s} p0[a]*p1[c]  -> [B, 19].

Sharding: pure data parallel over the batch across 8 NeuronCores.
Host-side prep: shard + transpose x to [784, B/8] per core so the device
DMA streams are contiguous; weights replicated (cast to bf16 on host).

Device-side per core (BC = 8192 samples per net):
  per round (512 samples of one net):
    - SWDGE cast-DMA loads xT tile [128f, 7fc, 512b] (fp32 HBM -> bf16 SBUF)
    - mm1: hT[j, b] += W1[f, j].T @ xT[f, b]   (PSUM, 4 j-chunks x 7 f-chunks)
    - bias+relu+cast -> SBUF bf16 (split across DVE and ACT)
    - mm2: l[b, 10]  += hT[j, b].T @ W2[j, 10] (PSUM, accumulate 4 j-chunks)
    - exp (ACT, PSUM -> SBUF block buffer), softmax denominator deferred
  per block (8 pair-rounds = 4096 sample pairs):
    - scale by exp(b2), Z sums, digit-sum convolution, normalize (DVE), DMA out
"""

import sys
import types

import numpy as np
import concourse.bass as bass
import concourse.bacc as bacc
import concourse.mybir as mybir
import concourse.tile as tile
from concourse.bass_utils import run_bass_kernel_spmd

F32 = mybir.dt.float32
MMDT = mybir.dt.float16  # matmul dtype: fp16 = bf16 speed, 4x less rounding error

NCORES = 8
B = 65536
BC = B // NCORES            # samples per core per net
F = 784                     # input features
HID = 512
NCLS = 10
NSUM = 19
FCH_FULL = F // 128         # 6 full feature chunks
FT = F - FCH_FULL * 128     # 16 tail features
FCH = FCH_FULL + 1          # 7 chunks
JCH = HID // 128            # 4 hidden chunks
# The 16 tail features are replicated at 4 32-row strips (rows 768+32k..+16)
# so the 4 per-j-chunk tail matmuls sit in distinct PE row-groups and run
# concurrently (row tiling).
FPAD = FCH * 128  # padded rows: 6*128 full chunks + 4 strips * 32 = 896
RN = 512                    # samples per compute round
XR = 1024                   # samples per x-load tile
XRR = XR // RN              # compute rounds per x tile
NR = BC // RN               # 16 pair-rounds per core
NBLK = 8                    # conv blocks; NR % NBLK == 0
RPB = NR // NBLK            # pair-rounds per block
GPB = RPB * (RN // 128)     # 128-sample groups per block


def pad_rows(a, dtype):
    """[F, ...] -> [FPAD, ...] with tail rows replicated at 4 32-row strips."""
    out = np.zeros((FPAD,) + a.shape[1:], dtype=dtype)
    out[: FCH_FULL * 128] = a[: FCH_FULL * 128]
    for k in range(JCH):
        base = FCH_FULL * 128 + 32 * k
        out[base : base + FT] = a[FCH_FULL * 128 :]
    return out


def build_nc():
    nc = bacc.Bacc("TRN2", target_bir_lowering=False, debug=False, num_devices=NCORES)

    xt = [
        nc.dram_tensor(f"xt{n}", [FPAD, BC], F32, kind="ExternalInput")
        for n in range(2)
    ]
    # weights arrive host-prepacked in their on-chip layouts so every const
    # DMA is one contiguous line per partition (cheap descriptor generation)
    w1 = [
        nc.dram_tensor(f"w1_{n}", [128, FCH, HID], MMDT, kind="ExternalInput")
        for n in range(2)
    ]
    w2 = [
        nc.dram_tensor(f"w2_{n}", [128, JCH, NCLS], MMDT, kind="ExternalInput")
        for n in range(2)
    ]
    # cpack[p] = [b1_0 (4), b1_1 (4), eb2_0 (10), eb2_1 (10)]
    cpk = nc.dram_tensor("cpack", [128, 28], F32, kind="ExternalInput")
    out = nc.dram_tensor("out", [BC, NSUM], F32, kind="ExternalOutput")

    with tile.TileContext(nc) as tc:
        with (
            tc.tile_pool(name="consts", bufs=1) as consts,
            tc.tile_pool(name="xt", bufs=4) as xt_pool,
            tc.tile_pool(name="ht", bufs=3) as ht_pool,
            tc.tile_pool(name="eblk", bufs=3) as e_pool,
            tc.tile_pool(name="conv", bufs=1) as conv_pool,
            tc.tile_pool(name="outp", bufs=2) as out_pool,
            tc.tile_pool(name="hpsum", bufs=6, space="PSUM") as h_pool,
            tc.tile_pool(name="lpsum", bufs=2, space="PSUM") as l_pool,
        ):
            # ---- constants (HWDGE; identity layouts) ----
            w1_sb = []
            w2_sb = []
            for n in range(2):
                w = consts.tile([128, FCH, HID], MMDT, name=f"w1sb{n}", tag=f"w1sb{n}")
                nc.sync.dma_start(out=w, in_=w1[n].ap())
                w1_sb.append(w)
            for n in range(2):
                w2t = consts.tile([128, JCH, NCLS], MMDT, name=f"w2sb{n}", tag=f"w2sb{n}")
                nc.sync.dma_start(out=w2t, in_=w2[n].ap())
                w2_sb.append(w2t)
            cpack = consts.tile([128, 28], F32, name="cpack", tag="cpack")
            nc.sync.dma_start(out=cpack, in_=cpk.ap())
            b1_sb = [cpack[:, 0:JCH], cpack[:, JCH : 2 * JCH]]
            eb2_sb = [
                cpack[:, 2 * JCH : 2 * JCH + NCLS],
                cpack[:, 2 * JCH + NCLS : 2 * JCH + 2 * NCLS],
            ]

            # ---- conv over a range of groups ----
            def emit_conv(ev, gc, row0):
                # ev: e slice [128, 2, gc, NCLS]; writes out rows row0..
                for n in range(2):
                    nc.vector.tensor_tensor(
                        out=ev[:, n, :, :],
                        in0=ev[:, n, :, :],
                        in1=eb2_sb[n][:, None, :].to_broadcast([128, gc, NCLS]),
                        op=mybir.AluOpType.mult,
                    )
                z = conv_pool.tile([128, 2, gc], F32, name="z", tag="z")
                for n in range(2):
                    nc.vector.reduce_sum(
                        out=z[:, n, :],
                        in_=ev[:, n, :, :],
                        axis=mybir.AxisListType.X,
                    )
                rz = conv_pool.tile([128, gc], F32, name="rz", tag="rz")
                nc.vector.tensor_tensor(
                    out=rz, in0=z[:, 0, :], in1=z[:, 1, :], op=mybir.AluOpType.mult
                )
                nc.vector.reciprocal(out=rz, in_=rz)
                prods = conv_pool.tile(
                    [128, gc, NCLS, NCLS], F32, name="prods", tag="prods"
                )
                for a in range(NCLS):
                    nc.vector.tensor_tensor(
                        out=prods[:, :, a, :],
                        in0=ev[:, 1, :, :],
                        in1=ev[:, 0, :, a : a + 1].to_broadcast([128, gc, NCLS]),
                        op=mybir.AluOpType.mult,
                    )
                acc = out_pool.tile([128, gc, NSUM], F32, name="acc", tag="acc")
                nc.vector.memset(acc, 0.0)
                for a in range(NCLS):
                    nc.vector.tensor_tensor(
                        out=acc[:, :, a : a + NCLS],
                        in0=acc[:, :, a : a + NCLS],
                        in1=prods[:, :, a, :],
                        op=mybir.AluOpType.add,
                    )
                nc.vector.tensor_tensor(
                    out=acc,
                    in0=acc,
                    in1=rz[:, :, None].to_broadcast([128, gc, NSUM]),
                    op=mybir.AluOpType.mult,
                )
                nc.sync.dma_start(
                    out=out.ap()[row0 : row0 + gc * 128, :].rearrange(
                        "(g p) s -> p g s", p=128
                    ),
                    in_=acc,
                )

            def conv_for_block(blk, e_t):
                if blk == NBLK - 1 and GPB % 2 == 0:
                    # final block in halves: less conv exposed at the end
                    hg = GPB // 2
                    emit_conv(e_t[:, :, 0:hg, :], hg, blk * GPB * 128)
                    emit_conv(e_t[:, :, hg:GPB, :], hg, blk * GPB * 128 + hg * 128)
                else:
                    emit_conv(e_t, GPB, blk * GPB * 128)

            # mm2 + exp for a finished round; emitted mid-way through the NEXT
            # round's mm1 so its relu inputs are complete (no PE wait)
            def emit_mm2_exp(pv):
                pn, pht, pblk, prr, pe_t = pv
                pl = l_pool.tile([128, RN // 128, NCLS], F32, name="pl", tag="pl")
                for bc in range(RN // 128):
                    for jc in range(JCH):
                        nc.tensor.matmul(
                            out=pl[:, bc, :],
                            lhsT=pht[:, jc, bc * 128 : (bc + 1) * 128],
                            rhs=w2_sb[pn][:, jc, :],
                            start=(jc == 0),
                            stop=(jc == JCH - 1),
                        )
                nc.scalar.activation(
                    out=pe_t[:, pn, prr * 4 : (prr + 1) * 4, :],
                    in_=pl[:, :, :],
                    func=mybir.ActivationFunctionType.Exp,
                )
                if pn == 1 and prr == RPB - 1:
                    conv_for_block(pblk, pe_t)

            # ---- main loop (mm2/exp software-pipelined one round behind) ----
            xtiles = {}
            e_tiles = {}
            prev = None
            for blk in range(NBLK):
                # exp values for this block: [128, net, group, cls]
                e_tiles[blk] = e_pool.tile(
                    [128, 2, GPB, NCLS], F32, name="eblk", tag="eblk"
                )
                for rr in range(RPB):
                    r = blk * RPB + rr
                    for n in range(2):
                        # x tiles hold XR samples (several compute rounds):
                        # bigger DMA lines (XR*4B per partition) + fewer DMAs
                        if r % XRR == 0:
                            xtiles[n] = xt_pool.tile(
                                [128, FCH, XR], MMDT, name="xt", tag="xt"
                            )
                            xsrc = xt[n].ap()[:, r * RN : r * RN + XR]
                            # tail chunk (6) first: each compute round begins
                            # with the row-tiled tail matmuls
                            for c0, c1 in ((6, 7), (0, 2), (2, 4), (4, 6)):
                                nc.gpsimd.dma_start(
                                    out=xtiles[n][:, c0:c1, :],
                                    in_=xsrc[c0 * 128 : c1 * 128, :].rearrange(
                                        "(c p) b -> p c b", p=128
                                    ),
                                )
                        xtile = xtiles[n][:, :, (r % XRR) * RN : (r % XRR + 1) * RN]

                        ph = [
                            h_pool.tile([128, RN], F32, name="ph", tag="ph")
                            for i in range(JCH)
                        ]
                        # tail features first: 4 concurrent row-tiled K=16
                        # matmuls (start=True initializes every psum element)
                        for jc in range(JCH):
                            p0 = 32 * jc
                            nc.tensor.matmul(
                                out=ph[jc][:, :],
                                lhsT=w1_sb[n][
                                    p0 : p0 + FT, FCH_FULL, jc * 128 : (jc + 1) * 128
                                ],
                                rhs=xtile[p0 : p0 + FT, FCH_FULL, :],
                                start=True,
                                stop=False,
                                tile_position=(p0, 0),
                            )
                        ht = ht_pool.tile([128, JCH, RN], MMDT, name="ht", tag="ht")
                        # jc-major: each j-chunk finishes early so its relu
                        # (all on ACT) drains the psum slot while the round runs
                        for jc in range(JCH):
                            for fc in range(FCH_FULL):
                                nc.tensor.matmul(
                                    out=ph[jc][:, :],
                                    lhsT=w1_sb[n][:, fc, jc * 128 : (jc + 1) * 128],
                                    rhs=xtile[:, fc, :],
                                    start=False,
                                    stop=(fc == FCH_FULL - 1),
                                )
                            nc.scalar.activation(
                                out=ht[:, jc, :],
                                in_=ph[jc][:, :],
                                func=mybir.ActivationFunctionType.Relu,
                                bias=b1_sb[n][:, jc : jc + 1],
                                scale=1.0,
                            )
                            if jc == 1 and prev is not None:
                                emit_mm2_exp(prev)
                        prev = (n, ht, blk, rr, e_tiles[blk])
            emit_mm2_exp(prev)

    nc.compile()
    return nc


_NC_CACHE = {}


def _get_nc():
    if "nc" not in _NC_CACHE:
        _NC_CACHE["nc"] = build_nc()
    return _NC_CACHE["nc"]


def _install_ntff_hook():
    """Shim antenv.axon_hooks (absent in this image) so trace=True can work."""
    try:
        import antenv

        if hasattr(antenv, "axon_hooks"):
            return
        from trn_agent_boot.trn_boot import _ntff_profile_via_ctypes

        mod = types.ModuleType("antenv.axon_hooks")
        holder = {"hook": _ntff_profile_via_ctypes("/opt/axon/libaxon_pjrt.so")}
        mod.set_axon_ntff_profile_hook = lambda h: holder.__setitem__("hook", h)
        mod.get_axon_ntff_profile_hook = lambda: holder["hook"]
        sys.modules["antenv.axon_hooks"] = mod
        antenv.axon_hooks = mod
    except Exception:
        pass


def kernel(x, W1_0, b1_0, W2_0, b2_0, W1_1, b1_1, W2_1, b2_1, _trace=False):
    x = np.asarray(x, dtype=np.float32)

    xf = x.reshape(2, B, F)
    weights = {}
    cpack = np.zeros((128, 28), np.float32)
    for n, (W1n, b1n, W2n, b2n) in enumerate(
        [(W1_0, b1_0, W2_0, b2_0), (W1_1, b1_1, W2_1, b2_1)]
    ):
        w1p = pad_rows(np.asarray(W1n, np.float32).astype(np.float16), np.float16)
        weights[f"w1_{n}"] = np.ascontiguousarray(
            w1p.reshape(FCH, 128, HID).transpose(1, 0, 2)
        )
        w2 = np.asarray(W2n, np.float32).astype(np.float16)
        weights[f"w2_{n}"] = np.ascontiguousarray(
            w2.reshape(JCH, 128, NCLS).transpose(1, 0, 2)
        )
        cpack[:, n * JCH : (n + 1) * JCH] = (
            np.asarray(b1n, np.float32).reshape(JCH, 128).T
        )
        cpack[:, 2 * JCH + n * NCLS : 2 * JCH + (n + 1) * NCLS] = np.exp(
            np.asarray(b2n, np.float32)
        )[None, :]
    weights["cpack"] = cpack

    in_maps = []
    for c in range(NCORES):
        m = dict(weights)
        for n in range(2):
            shard = xf[n, c * BC : (c + 1) * BC, :]  # [BC, F]
            m[f"xt{n}"] = pad_rows(np.ascontiguousarray(shard.T), np.float32)
        in_maps.append(m)

    nc = _get_nc()
    if _trace:
        _install_ntff_hook()
    res = run_bass_kernel_spmd(nc, in_maps, list(range(NCORES)), trace=_trace)
    pieces = [res.results[c]["out"] for c in range(NCORES)]
    full = np.concatenate(pieces, axis=0).astype(np.float32)
    if _trace:
        return full, res
    return full
